# revision 1
# baseline (speedup 1.0000x reference)
"""Trainium2 Bass kernel for nn_Decoder (4-layer transformer decoder).

Sharding: 8 cores = 4 batches x 2 token-halves. Core (b, r) owns token blocks
{r, r+2, r+4, r+6} (128 tokens each, interleaved for causal load balance).
Within a pair, K/V are exchanged per layer via pair AllGather (bf16).

Layout: activations kept transposed (xT: [DM on partitions, tokens free]).
Projections / FFN / Wo run in float32r (TF32-like, full PE rate at N>=512).
Attention core (scores, exp, AV) runs in bf16. Per-token stats (layernorm,
softmax denominator) are computed with ones-matmuls on the PE and broadcast
back across partitions with K=1 ones-matmuls.

Self-attention causal structure is made core-uniform by padding each key
block's query window to N(kb) = 512 - 128*ceil((kb-1)/2); the first 128-col
slab of each window gets a host-supplied 0/1 multiplicative mask (applied
after exp), which also absorbs the fully-dead slabs on even-half cores.
"""

import math

import numpy as np
import ml_dtypes

# Problem constants (hardcoded; must match the harness problem).
L, DM, H, DK, DV, DFF = 4, 1024, 16, 64, 64, 4096
B, T = 4, 1024
EPS = 1e-5

P = 128
TOK = 512                      # tokens owned per core
ND = DM // P                   # 8 dm partition-tiles
NKB = T // P                   # 8 key blocks
NTB = TOK // P                 # 4 own token blocks
HP = H // 2                    # 8 head pairs
NF = DFF // P                  # 32 ffn row tiles
J0 = [max(0, math.ceil((kb - 1) / 2)) for kb in range(NKB)]
GPOS = [kb // 2 if kb % 2 == 0 else 4 + kb // 2 for kb in range(NKB)]

_BUILT = {}


def _build(num_devices=8, use_ag=True, self_causal=True):
    import concourse.bass as bass
    import concourse.tile as tile
    from concourse import bacc, mybir
    from contextlib import ExitStack

    dt = mybir.dt
    f32, f32r, bf16 = dt.float32, dt.float32r, dt.bfloat16
    AF = mybir.ActivationFunctionType
    OP = mybir.AluOpType
    RG = [[0, 1], [2, 3], [4, 5], [6, 7]]

    nc = bacc.Bacc("TRN2", target_bir_lowering=False, debug=False, num_devices=num_devices)

    # ---- I/O ----
    xT_ext = nc.dram_tensor("xT", [DM, TOK], f32, kind="ExternalInput").ap()
    encT_ext = nc.dram_tensor("encT", [DM, TOK], bf16, kind="ExternalInput").ap()
    smask_ext = nc.dram_tensor("smask", [NKB, P, P], bf16, kind="ExternalInput").ap()
    w_ext = {}
    for name, shp in [
        ("self_Wq", [L, DM, H * DK]), ("self_Wk", [L, DM, H * DK]),
        ("self_Wv", [L, DM, H * DV]),
        ("cross_Wq", [L, DM, H * DK]), ("cross_Wk", [L, DM, H * DK]),
        ("cross_Wv", [L, DM, H * DV]),
    ]:
        w_ext[name] = nc.dram_tensor(name, shp, bf16, kind="ExternalInput").ap()
    for name, shp in [
        ("self_Wo", [L, H * DV, DM]), ("cross_Wo", [L, H * DV, DM]),
        ("ffn_W1", [L, DM, DFF]), ("ffn_W2", [L, DFF, DM]),
    ]:
        w_ext[name] = nc.dram_tensor(name, shp, f32, kind="ExternalInput").ap()
    yT_ext = nc.dram_tensor("yT", [DM, TOK], f32, kind="ExternalOutput").ap()

    with tile.TileContext(nc) as tc, ExitStack() as stack:
        pers = stack.enter_context(tc.tile_pool(name="pers", bufs=1))
        dram = stack.enter_context(tc.tile_pool(name="dram", bufs=1, space="DRAM"))

        # constants
        ones_col_f = pers.tile([P, 1], f32, tag="ones_col_f")
        nc.vector.memset(ones_col_f[:], 1.0)
        ones_col = pers.tile([P, 1], f32r, tag="ones_col")
        nc.scalar.copy(ones_col[:], ones_col_f[:])
        ones_row_f = pers.tile([1, P], f32, tag="ones_row_f")
        nc.vector.memset(ones_row_f[:], 1.0)
        ones_row = pers.tile([1, P], f32r, tag="ones_row")
        nc.scalar.copy(ones_row[:], ones_row_f[:])
        eps_t = pers.tile([1, 1], f32, tag="eps_t")
        nc.vector.memset(eps_t[:], EPS)

        # resident activations
        x_cur = pers.tile([P, ND, TOK], f32r, tag="x", bufs=2, name="x0")
        nc.sync.dma_start(
            x_cur[:], xT_ext.rearrange("(o p) t -> p o t", p=P).bitcast(f32r))
        enc_sb = pers.tile([P, ND, TOK], bf16, tag="enc")
        nc.sync.dma_start(
            enc_sb[:], encT_ext.rearrange("(o p) t -> p o t", p=P))
        smask_sb = pers.tile([P, NKB, P], bf16, tag="smask")
        nc.sync.dma_start(smask_sb[:], smask_ext.rearrange("k p q -> p k q"))

        def load_w8(pool, ext_l, half, nm):
            """[DM, 1024] weight half -> [128, ND, 512] f32r tile."""
            w = pool.tile([P, ND, 512], f32r, tag="whalf", bufs=2, name=f"w_{nm}")
            src = ext_l.rearrange("(o p) m -> p o m", p=P)
            for d in range(ND):
                nc.sync.dma_start(
                    w[:, d, :],
                    src[:, d, half * 512:(half + 1) * 512].bitcast(f32r))
            return w

        def load_w8b(pool, ext_l, half, nm):
            """[DM, 1024] bf16 weight half -> [128, ND, 512] bf16 tile."""
            w = pool.tile([P, ND, 512], bf16, tag="whb", bufs=3, name=f"wb_{nm}")
            src = ext_l.rearrange("(o p) m -> p o m", p=P)
            for d in range(ND):
                nc.sync.dma_start(
                    w[:, d, :], src[:, d, half * 512:(half + 1) * 512])
            return w

        def cast_xb(ph, nm):
            """x_cur -> bf16 copy for the bf16 QKV projections."""
            xb = ph.tile([P, ND, TOK], bf16, tag="xb", bufs=1, name=f"xb_{nm}")
            for m in range(ND):
                nc.scalar.copy(xb[:, m, :], x_cur[:, m, :])
            return xb

        def projT(w_sb, xin, pools, consume, tiles=range(4)):
            """for m: psum = sum_d w_sb[:,d,m*128:+128].T @ xin[:,d,:] -> consume(m, ps)."""
            for m in tiles:
                ps = pools.tile([P, TOK], f32, tag="proj", bufs=2, name=f"pps_{m}")
                for d in range(ND):
                    nc.tensor.matmul(
                        ps[:], w_sb[:, d, m * P:(m + 1) * P], xin[:, d, :],
                        start=(d == 0), stop=(d == ND - 1))
                consume(m, ps)

        def k_proj(ph, pools, xin, wk_l, kt_bounce, nm):
            kto = ph.tile([P, ND, TOK], bf16, tag="kto", name=f"kto_{nm}")
            for half in range(2):
                wk = load_w8b(ph, wk_l, half, f"k{nm}{half}")

                def eat_k(m, ps, half=half):
                    nc.scalar.copy(kto[:, half * 4 + m, :], ps[:])
                projT(wk, xin, pools, eat_k)
            nc.sync.dma_start(kt_bounce.rearrange("(o p) t -> p o t", p=P), kto[:])

        def v_proj(ph, pools, xin, wv_l, va_bounce, nm):
            vao = ph.tile([P, NTB, H, DV + 1], bf16, tag="vao", name=f"vao_{nm}")
            for half in range(2):
                wv = load_w8b(ph, wv_l, half, f"v{nm}{half}")
                for tb in range(NTB):
                    ps = pools.tile([P, TOK], f32, tag="proj", bufs=2,
                                    name=f"vps_{nm}{half}{tb}")
                    for d in range(ND):
                        nc.tensor.matmul(
                            ps[:], xin[:, d, tb * P:(tb + 1) * P], wv[:, d, :],
                            start=(d == 0), stop=(d == ND - 1))
                    nc.scalar.copy(
                        vao[:, tb, half * 8:(half + 1) * 8, 0:DV],
                        ps.rearrange("p (h v) -> p h v", h=8))
            nc.vector.memset(vao[:, :, :, DV:DV + 1], 1.0)
            nc.sync.dma_start(
                va_bounce.rearrange("(tb p) c -> p tb c", p=P),
                vao.rearrange("p tb h v -> p tb (h v)"))

        def q_proj(ph, pools, xin, wq_l, nm):
            qt = pers.tile([P, ND, TOK], bf16, tag="qt", name=f"qt_{nm}")
            for half in range(2):
                wq = load_w8b(ph, wq_l, half, f"q{nm}{half}")

                def eat_q(m, ps, half=half):
                    nc.scalar.copy(qt[:, half * 4 + m, :], ps[:])
                projT(wq, xin, pools, eat_q)
            return qt

        def load_kv(ph, ktg_d, vg_d, nm):
            ktg = ph.tile([P, HP, 2, TOK], bf16, tag="ktg", name=f"ktg_{nm}")
            for s in range(2):
                nc.sync.dma_start(
                    ktg[:, :, s, :],
                    ktg_d[s * DM:(s + 1) * DM].rearrange("(o p) t -> p o t", p=P))
            vg = ph.tile([P, NKB, H, DV + 1], bf16, tag="vg", name=f"vg_{nm}")
            nc.sync.dma_start(
                vg[:], vg_d.rearrange("(s tb p) c -> p (s tb) c", s=2, p=P)
                .rearrange("p b (h v) -> p b h v", h=H))
            return ktg, vg

        def attention(ph, aps, qt, ktg, vg, masked, nm):
            """Gathered KV (SBUF) -> normalized ctx_sb [P, ND, TOK] f32r."""
            ctx_sb = pers.tile([P, ND, TOK], f32r, tag="ctxs", name=f"ctx_{nm}")
            for p in range(HP):
                cps = [aps.tile([DV + 1, TOK], f32, tag="ctxps", bufs=2,
                                name=f"cps_{nm}{p}{h}") for h in range(2)]
                for kb in range(NKB):
                    qo = J0[kb] * P if masked else 0
                    c = GPOS[kb]
                    es = ph.tile([P, 2, TOK], bf16, tag="es", bufs=3,
                                 name=f"es_{nm}{p}{kb}")
                    for h in range(2):
                        sc = aps.tile([P, TOK], f32, tag="sc", bufs=3,
                                      name=f"sc_{nm}{p}{kb}{h}")
                        nc.tensor.matmul(
                            sc[:, qo:],
                            ktg[h * DV:(h + 1) * DV, p, c // 4,
                                (c % 4) * P:(c % 4 + 1) * P],
                            qt[h * DV:(h + 1) * DV, p, qo:],
                            start=True, stop=True)
                        nc.scalar.activation(
                            es[:, h, qo:], sc[:, qo:],
                            AF.Exp, scale=1.0 / math.sqrt(DK))
                    if masked:
                        nc.vector.tensor_tensor(
                            es[:, :, qo:qo + P], es[:, :, qo:qo + P],
                            smask_sb[:, kb, None, :].to_broadcast([P, 2, P]),
                            OP.mult)
                    for h in range(2):
                        nc.tensor.matmul(
                            cps[h][:, qo:], vg[:, c, 2 * p + h, :],
                            es[:, h, qo:], start=(kb == 0), stop=(kb == NKB - 1))
                for h in range(2):
                    rec = pers.tile([1, TOK], f32r, tag="rec", bufs=2,
                                    name=f"rec_{nm}{p}{h}")
                    with nc.allow_low_precision(reason="f32r softmax denom"):
                        nc.vector.reciprocal(rec[:], cps[h][DV:DV + 1, :])
                    bc = aps.tile([P, TOK], f32, tag="bcps", bufs=1,
                                  name=f"bc_{nm}{p}{h}")
                    nc.tensor.matmul(bc[:], ones_row[:], rec[:],
                                     start=True, stop=True)
                    nc.vector.tensor_copy(ctx_sb[h * DV:(h + 1) * DV, p, :],
                                          cps[h][0:DV, :])
                    nc.vector.tensor_tensor(
                        ctx_sb[h * DV:(h + 1) * DV, p, :],
                        ctx_sb[h * DV:(h + 1) * DV, p, :], bc[0:DV, :],
                        OP.mult)
            return ctx_sb

        def residual_add(get_in, nm):
            """xn = in + x_cur (per dm-tile)."""
            xn = pers.tile([P, ND, TOK], f32r, tag="x", bufs=2, name=f"x_{nm}")
            for m in range(ND):
                nc.vector.tensor_tensor(xn[:, m, :], get_in(m), x_cur[:, m, :],
                                        OP.add)
            return xn

        def ln_apply(xn, nm):
            """In-place layernorm of xn across the DM (partition-tiled) axis."""
            nonlocal x_cur
            with tc.tile_pool(name=f"lps_{nm}", bufs=1, space="PSUM") as lps:
                ssum = lps.tile([1, TOK], f32, tag="stsum", name=f"ssum_{nm}")
                ssq = lps.tile([1, TOK], f32, tag="stsq", name=f"ssq_{nm}")
                for m in range(ND):
                    sq = pers.tile([P, TOK], f32r, tag="sq", bufs=2,
                                   name=f"sq_{nm}{m}")
                    nc.scalar.square(sq[:], xn[:, m, :])
                    nc.tensor.matmul(ssum[:], ones_col[:], xn[:, m, :],
                                     start=(m == 0), stop=(m == ND - 1))
                    nc.tensor.matmul(ssq[:], ones_col[:], sq[:],
                                     start=(m == 0), stop=(m == ND - 1))
                mean = pers.tile([1, TOK], f32r, tag="mean", name=f"mean_{nm}")
                nc.vector.tensor_scalar_mul(mean[:], ssum[:], 1.0 / DM)
                es2 = pers.tile([1, TOK], f32, tag="es2", name=f"es2_{nm}")
                nc.vector.tensor_scalar_mul(es2[:], ssq[:], 1.0 / DM)
                msq = pers.tile([1, TOK], f32, tag="msq", name=f"msq_{nm}")
                nc.scalar.square(msq[:], mean[:])
                var = pers.tile([1, TOK], f32, tag="var", name=f"var_{nm}")
                nc.vector.tensor_tensor(var[:], es2[:], msq[:], OP.subtract)
                sS = pers.tile([1, TOK], f32r, tag="sS", name=f"sS_{nm}")
                nc.scalar.activation(sS[:], var[:], AF.Abs_reciprocal_sqrt,
                                     bias=eps_t[:])
                Mb = lps.tile([P, TOK], f32, tag="Mb", name=f"Mb_{nm}")
                nc.tensor.matmul(Mb[:], ones_row[:], mean[:], start=True, stop=True)
                Mbs = pers.tile([P, TOK], f32, tag="Mbs", name=f"Mbs_{nm}")
                nc.scalar.copy(Mbs[:], Mb[:])
                # x - mean overlaps the inv-std chain; all-SBUF ops hit DVE 2x
                for m in range(ND):
                    nc.vector.tensor_tensor(xn[:, m, :], xn[:, m, :], Mbs[:],
                                            OP.subtract)
                Sb = lps.tile([P, TOK], f32, tag="Sb", name=f"Sb_{nm}")
                nc.tensor.matmul(Sb[:], ones_row[:], sS[:], start=True, stop=True)
                Sbs = pers.tile([P, TOK], f32, tag="Sbs", name=f"Sbs_{nm}")
                nc.scalar.copy(Sbs[:], Sb[:])
                for m in range(ND):
                    nc.vector.tensor_tensor(xn[:, m, :], xn[:, m, :], Sbs[:],
                                            OP.mult)
            x_cur = xn

        def wo_add(ph, aps, wo_l, ctx_sb, nm):
            """Wo matmuls + residual add, inside the caller's merged scope."""
            whs = [load_w8(ph, wo_l, half, f"o{nm}{half}") for half in range(2)]
            xn = pers.tile([P, ND, TOK], f32r, tag="x", bufs=2, name=f"x_{nm}")
            for m in range(ND):
                ps = aps.tile([P, TOK], f32, tag="proj", bufs=2,
                              name=f"wops_{nm}{m}")
                half, mm = divmod(m, 4)
                for v in range(ND):
                    nc.tensor.matmul(
                        ps[:], whs[half][:, v, mm * P:(mm + 1) * P],
                        ctx_sb[:, v, :], start=(v == 0), stop=(v == ND - 1))
                nc.vector.tensor_tensor(xn[:, m, :], ps[:], x_cur[:, m, :],
                                        OP.add)
            return xn

        def dram_kt(nm):
            ktb = dram.tile([DM, TOK], bf16, tag="ktb", bufs=4, name=f"ktb_{nm}")
            ktg = dram.tile([2 * DM, TOK], bf16, tag="ktg", bufs=4,
                            name=f"ktg_{nm}")
            return ktb, ktg

        def dram_va(nm):
            vab = dram.tile([TOK, H * (DV + 1)], bf16, tag="vab", bufs=4,
                            name=f"vab_{nm}")
            vag = dram.tile([2 * TOK, H * (DV + 1)], bf16, tag="vag", bufs=4,
                            name=f"vag_{nm}")
            return vab, vag

        def ag1(src, dst, rows):
            if num_devices == 1 or not use_ag:
                for s in range(2):
                    nc.sync.dma_start(dst[s * rows:(s + 1) * rows], src[:])
                return
            nc.gpsimd.collective_compute(
                "AllGather", mybir.AluOpType.bypass, replica_groups=RG,
                ins=[src.opt()], outs=[dst.opt()])

        cv_pre = {}
        for l in range(L):
            sktb, sktg = dram_kt(f"s{l}")
            svab, svag = dram_va(f"s{l}")
            cktb, cktg = dram_kt(f"c{l}")
            if l in cv_pre:
                cvab, cvag = cv_pre[l]
            else:
                cvab, cvag = dram_va(f"c{l}")
            # Self sublayer, one merged scope. Cross-K (and for l=0 cross-V)
            # projections are emitted after the attention loops so they
            # gap-fill the PE while the scalar engine works through the exps.
            with tc.tile_pool(name=f"ph1_{l}", bufs=1) as ph, \
                 tc.tile_pool(name=f"ps1_{l}", bufs=1, space="PSUM") as aps:
                xb = cast_xb(ph, f"s{l}")
                k_proj(ph, aps, xb, w_ext["self_Wk"][l], sktb, f"s{l}")
                ag1(sktb, sktg, DM)
                v_proj(ph, aps, xb, w_ext["self_Wv"][l], svab, f"s{l}")
                ag1(svab, svag, TOK)
                qt = q_proj(ph, aps, xb, w_ext["self_Wq"][l], f"s{l}")
                sktg_sb, svag_sb = load_kv(ph, sktg, svag, f"s{l}")
                ctx = attention(ph, aps, qt, sktg_sb, svag_sb, self_causal,
                                f"s{l}")
                k_proj(ph, aps, enc_sb, w_ext["cross_Wk"][l], cktb, f"c{l}")
                ag1(cktb, cktg, DM)
                v_proj(ph, aps, enc_sb, w_ext["cross_Wv"][l], cvab, f"c{l}")
                ag1(cvab, cvag, TOK)
                xn = wo_add(ph, aps, w_ext["self_Wo"][l], ctx, f"s{l}")
            ln_apply(xn, f"s{l}")

            # cross sublayer
            with tc.tile_pool(name=f"ph4_{l}", bufs=1) as ph, \
                 tc.tile_pool(name=f"ps4_{l}", bufs=1, space="PSUM") as aps:
                cktg_sb, cvag_sb = load_kv(ph, cktg, cvag, f"c{l}")
                xb = cast_xb(ph, f"c{l}")
                qtc = q_proj(ph, aps, xb, w_ext["cross_Wq"][l], f"c{l}")
                ctx = attention(ph, aps, qtc, cktg_sb, cvag_sb, False, f"c{l}")
                xn = wo_add(ph, aps, w_ext["cross_Wo"][l], ctx, f"c{l}")
            ln_apply(xn, f"c{l}")

            # FFN
            with tc.tile_pool(name=f"ph6_{l}", bufs=1) as ph:
                h_sb = ph.tile([P, NF, TOK], f32r, tag="h", name=f"h_{l}")
                with tc.tile_pool(name=f"ps6_{l}", bufs=1, space="PSUM") as pools:
                    w1r = w_ext["ffn_W1"][l].rearrange("(o p) f -> p o f", p=P)
                    for c in range(DFF // 512):
                        w1c = ph.tile([P, ND, 512], f32r, tag="w1c", bufs=2,
                                      name=f"w1c_{l}{c}")
                        if c == 0:
                            # split first chunk across queues to cut latency
                            for d in range(ND):
                                nc.sync.dma_start(
                                    w1c[:, d, :],
                                    w1r[:, d, 0:512].bitcast(f32r))
                        else:
                            nc.sync.dma_start(
                                w1c[:], w1r[:, :, c * 512:(c + 1) * 512].bitcast(f32r))
                        for ft in range(4):
                            ps = pools.tile([P, TOK], f32, tag="hps", bufs=2,
                                            name=f"hps_{l}{c}{ft}")
                            for d in range(ND):
                                nc.tensor.matmul(
                                    ps[:], w1c[:, d, ft * P:(ft + 1) * P],
                                    x_cur[:, d, :],
                                    start=(d == 0), stop=(d == ND - 1))
                            nc.scalar.activation(h_sb[:, c * 4 + ft, :], ps[:],
                                                 AF.Relu)
                with tc.tile_pool(name=f"ps7_{l}", bufs=1, space="PSUM") as pools:
                    yps = [pools.tile([P, TOK], f32, tag=f"y{m}",
                                      name=f"yps_{l}{m}") for m in range(ND)]
                    w2r = w_ext["ffn_W2"][l].rearrange("(f p) m -> p f m", p=P)
                    for f in range(NF):
                        w2f = ph.tile([P, DM], f32r, tag="w2f", bufs=3,
                                      name=f"w2f_{l}{f}")
                        nc.sync.dma_start(w2f[:], w2r[:, f, :].bitcast(f32r))
                        for m in range(ND):
                            nc.tensor.matmul(
                                yps[m][:], w2f[:, m * P:(m + 1) * P],
                                h_sb[:, f, :],
                                start=(f == 0), stop=(f == NF - 1))
                    xn = residual_add(lambda m: yps[m][:], f"f{l}")
                ln_apply(xn, f"f{l}")

        yre = yT_ext.rearrange("(o p) t -> p o t", p=P).bitcast(f32r)
        for m in range(ND):
            nc.sync.dma_start(yre[:, m, :], x_cur[:, m, :])

    nc.compile()
    return nc


def _get_built(self_causal=True):
    if self_causal not in _BUILT:
        _BUILT[self_causal] = _build(self_causal=self_causal)
    return _BUILT[self_causal]


def _host_shard(inputs):
    """Build per-core input maps from full inputs."""
    dec = np.asarray(inputs["dec_inputs"], dtype=np.float32)
    enc = np.asarray(inputs["enc_outputs"], dtype=np.float32)
    smask_full = np.asarray(inputs["dec_self_attn_mask"]).astype(bool)
    cmask = np.asarray(inputs["dec_enc_attn_mask"]).astype(bool)
    assert not cmask.any(), "kernel assumes open cross-attention mask"

    weights = {}
    for k in ["self_Wq", "self_Wk", "self_Wv", "cross_Wq", "cross_Wk",
              "cross_Wv"]:
        weights[k] = np.ascontiguousarray(
            np.asarray(inputs[k], dtype=np.float32)).astype(ml_dtypes.bfloat16)
    for k in ["self_Wo", "cross_Wo", "ffn_W1", "ffn_W2"]:
        weights[k] = np.ascontiguousarray(np.asarray(inputs[k], dtype=np.float32))

    self_causal = smask_full.any()
    in_maps, row_sets = [], []
    for core in range(8):
        b, r = divmod(core, 2)
        rows = np.concatenate(
            [np.arange((2 * j + r) * P, (2 * j + r + 1) * P) for j in range(NTB)])
        row_sets.append((b, rows))
        xT = np.ascontiguousarray(dec[b][rows].T)
        encT = np.ascontiguousarray(enc[b][rows].T).astype(ml_dtypes.bfloat16)
        sm = np.ones((NKB, P, P), dtype=np.float32)
        mb = smask_full[b]
        if self_causal:
            for kb in range(NKB):
                qg0 = (2 * J0[kb] + r) * P
                blk = mb[qg0:qg0 + P, kb * P:(kb + 1) * P]     # [q, k]
                sm[kb] = (~blk.T).astype(np.float32)            # [k, q], 1=keep
                for j in range(NTB):
                    qg = (2 * j + r) * P
                    bj = mb[qg:qg + P, kb * P:(kb + 1) * P]
                    if j < J0[kb]:
                        assert bj.all(), "skipped block not fully masked"
                    elif j > J0[kb]:
                        assert not bj.any(), \
                            "unmasked block outside computed window"
        in_map = {"xT": xT, "encT": encT,
                  "smask": sm.astype(ml_dtypes.bfloat16)}
        in_map.update(weights)
        in_maps.append(in_map)
    return in_maps, row_sets, self_causal


def kernel(**inputs):
    from concourse.bass_utils import run_bass_kernel_spmd

    in_maps, row_sets, self_causal = _host_shard(inputs)
    nc = _get_built(self_causal)
    res = run_bass_kernel_spmd(nc, in_maps, core_ids=list(range(8)))
    out = np.empty((B, T, DM), dtype=np.float32)
    for core in range(8):
        b, rows = row_sets[core]
        out[b, rows, :] = res.results[core]["yT"].T
    return out



# revision 15
# speedup vs baseline: 3.7350x; 3.7350x over previous
"""Trainium2 Bass kernel for nn_Decoder (4-layer transformer decoder).

Sharding v2: 8 cores = 4 batches x 2 replicas. Each core computes its full
batch (all 1024 tokens); the pair redundancy removes every per-layer
collective (self K/V are local, cross K/V come from the static enc).
Weights are streamed host->device as per-core 1/8 bf16 chunks (16 MB/core
instead of a replicated ~184 MB/core) and reassembled on-device with one
8-way DRAM AllGather per layer, overlapped with compute.

Layout: activations transposed (xT: [DM on partitions, tokens free]).
All matmuls run in bf16 with f32 PSUM accumulation; the residual stream and
layernorm run in f32r. Per-token stats (layernorm, softmax denominator) are
computed with ones-matmuls on the PE and broadcast back across partitions
with K=1 ones-matmuls.

Self-attention causality: key block kb only attends queries q >= kb*128; the
diagonal 128-col slab gets a host-supplied 0/1 multiplicative mask applied
after exp.
"""

import math

import numpy as np
import ml_dtypes

# Problem constants (hardcoded; must match the harness problem).
L, DM, H, DK, DV, DFF = 4, 1024, 16, 64, 64, 4096
B, T = 4, 1024
EPS = 1e-5

P = 128
TOK = 1024                     # tokens per core (full batch)
ND = DM // P                   # 8 dm partition-tiles
NKB = T // P                   # 8 key blocks
HP = H // 2                    # 8 head pairs
NF = DFF // P                  # 32 ffn row tiles
LROWS = 16 * 1024              # 1024-wide rows per layer in the weight blob
CROWS = LROWS // 8             # rows per core per layer

# weight blob row offsets within a layer (units of [1024] rows)
WOFF = {
    "self_Wq": 0, "self_Wk": 1024, "self_Wv": 2048,
    "cross_Wq": 3072, "cross_Wk": 4096, "cross_Wv": 5120,
    "self_Wo": 6144, "cross_Wo": 7168,
    "ffn_W1": 8192, "ffn_W2": 12288,
}
WORDER = ["self_Wq", "self_Wk", "self_Wv", "cross_Wq", "cross_Wk",
          "cross_Wv", "self_Wo", "cross_Wo", "ffn_W1", "ffn_W2"]

_BUILT = {}


def _build(num_devices=8, use_ag=True, self_causal=True):
    import concourse.bass as bass
    import concourse.tile as tile
    from concourse import bacc, mybir
    from contextlib import ExitStack

    dt = mybir.dt
    f32, f32r, bf16 = dt.float32, dt.float32r, dt.bfloat16
    AF = mybir.ActivationFunctionType
    OP = mybir.AluOpType
    RG = [[0, 1, 2, 3, 4, 5, 6, 7]]

    nc = bacc.Bacc("TRN2", target_bir_lowering=False, debug=False, num_devices=num_devices)

    # ---- I/O ----
    xT_ext = nc.dram_tensor("xT", [DM, TOK], bf16, kind="ExternalInput").ap()
    encT_ext = nc.dram_tensor("encT", [DM, TOK], bf16, kind="ExternalInput").ap()
    smask_ext = nc.dram_tensor("smask", [NKB, P, P], bf16, kind="ExternalInput").ap()
    wch_ext = nc.dram_tensor("wchunk", [L * CROWS, 1024], bf16,
                             kind="ExternalInput").ap()
    yT_ext = nc.dram_tensor("yT", [DM, TOK], f32, kind="ExternalOutput").ap()

    with tile.TileContext(nc) as tc, ExitStack() as stack:
        pers = stack.enter_context(tc.tile_pool(name="pers", bufs=1))
        dram = stack.enter_context(tc.tile_pool(name="dram", bufs=1, space="DRAM"))

        # constants
        ones_col_f = pers.tile([P, 1], f32, tag="ones_col_f")
        nc.vector.memset(ones_col_f[:], 1.0)
        ones_col = pers.tile([P, 1], f32r, tag="ones_col")
        nc.scalar.copy(ones_col[:], ones_col_f[:])
        ones_row_f = pers.tile([1, P], f32, tag="ones_row_f")
        nc.vector.memset(ones_row_f[:], 1.0)
        ones_row = pers.tile([1, P], f32r, tag="ones_row")
        nc.scalar.copy(ones_row[:], ones_row_f[:])
        eps_t = pers.tile([1, 1], f32, tag="eps_t")
        nc.vector.memset(eps_t[:], EPS)

        # weight gather: per-layer 8-way AllGather of the bf16 blob chunks.
        # Collectives cannot read IO tensors, so bounce each chunk slice
        # through an Internal DRAM tile first.
        wall = []
        for l in range(L):
            wl = dram.tile([LROWS, 1024], bf16, tag="wall", bufs=L,
                           addr_space="Shared", name=f"wall_{l}")
            src = wch_ext[l * CROWS:(l + 1) * CROWS, :]
            if num_devices == 1 or not use_ag:
                for s in range(8):
                    nc.sync.dma_start(wl[s * CROWS:(s + 1) * CROWS, :], src)
            else:
                wb = dram.tile([CROWS, 1024], bf16, tag="wbnc", bufs=L,
                               name=f"wbnc_{l}")
                nc.sync.dma_start(wb[:], src)
                nc.gpsimd.collective_compute(
                    "AllGather", mybir.AluOpType.bypass, replica_groups=RG,
                    ins=[wb[:].opt()], outs=[wl[:].opt()])
            wall.append(wl)

        def wv2d(l, name):
            """2-D AP view [rows, 1024] of weight `name` in layer l's blob."""
            r0 = WOFF[name]
            nrows = {"ffn_W1": 4096, "ffn_W2": 4096}.get(name, 1024)
            return wall[l][r0:r0 + nrows, :]

        # resident activations (single buffer; residual adds are in-place)
        x_cur = pers.tile([P, ND, TOK], f32r, tag="x", bufs=1, name="x0")
        with tc.tile_pool(name="init", bufs=1) as ip:
            xb0 = ip.tile([P, ND, TOK], bf16, tag="xb0")
            nc.sync.dma_start(xb0[:], xT_ext.rearrange("(o p) t -> p o t", p=P))
            for m in range(ND):
                nc.scalar.copy(x_cur[:, m, :], xb0[:, m, :])
        enc_sb = pers.tile([P, ND, TOK], bf16, tag="enc")
        nc.sync.dma_start(
            enc_sb[:], encT_ext.rearrange("(o p) t -> p o t", p=P))
        smask_sb = pers.tile([P, NKB, P], bf16, tag="smask")
        nc.sync.dma_start(smask_sb[:], smask_ext.rearrange("k p q -> p k q"))

        def load_w8b(pool, src2d, half, nm):
            """[1024, 1024] bf16 weight half -> [128, ND, 512] bf16 tile."""
            w = pool.tile([P, ND, 512], bf16, tag="whb", bufs=2, name=f"wb_{nm}")
            src = src2d.rearrange("(o p) m -> p o m", p=P)
            for d in range(ND):
                nc.sync.dma_start(
                    w[:, d, :], src[:, d, half * 512:(half + 1) * 512])
            return w

        def cast_xb(ph, nm):
            """x_cur -> bf16 copy for the bf16 matmuls."""
            xb = ph.tile([P, ND, TOK], bf16, tag="xb", bufs=1, name=f"xb_{nm}")
            for m in range(ND):
                nc.scalar.copy(xb[:, m, :], x_cur[:, m, :])
            return xb

        def projT(w_sb, xin, pools, consume, tiles=range(4)):
            """for m: ps[half] = sum_d w_sb[:,d,m*128:+128].T @ xin[:,d,half]."""
            for m in tiles:
                pss = []
                for half in range(2):
                    ps = pools.tile([P, 512], f32, tag="proj", bufs=2,
                                    name=f"pps_{m}{half}")
                    for d in range(ND):
                        nc.tensor.matmul(
                            ps[:], w_sb[:, d, m * P:(m + 1) * P],
                            xin[:, d, half * 512:(half + 1) * 512],
                            start=(d == 0), stop=(d == ND - 1))
                    pss.append(ps)
                consume(m, pss)

        def kq_proj(ph, pools, xin, w2d, nm):
            """K^T or Q^T: [128 (2 heads x 64), HP, TOK] bf16."""
            out = ph.tile([P, HP, TOK], bf16, tag="kq", bufs=2, name=f"kq_{nm}")
            for half in range(2):
                w = load_w8b(ph, w2d, half, f"{nm}{half}")

                def eat(m, pss, half=half):
                    for h2 in range(2):
                        nc.scalar.copy(
                            out[:, half * 4 + m, h2 * 512:(h2 + 1) * 512],
                            pss[h2][:])
                projT(w, xin, pools, eat)
            return out

        def v_proj(ph, pools, xin, w2d, nm):
            """V (ones-augmented): [128 (key tok), NKB, H, DV+1] bf16."""
            vao = ph.tile([P, NKB, H, DV + 1], bf16, tag="vao", bufs=1,
                          name=f"vao_{nm}")
            for half in range(2):
                w = load_w8b(ph, w2d, half, f"v{nm}{half}")
                for tb in range(NKB):
                    ps = pools.tile([P, 512], f32, tag="proj", bufs=2,
                                    name=f"vps_{nm}{half}{tb}")
                    for d in range(ND):
                        nc.tensor.matmul(
                            ps[:], xin[:, d, tb * P:(tb + 1) * P], w[:, d, :],
                            start=(d == 0), stop=(d == ND - 1))
                    nc.scalar.copy(
                        vao[:, tb, half * 8:(half + 1) * 8, 0:DV],
                        ps.rearrange("p (h v) -> p h v", h=8))
            nc.vector.memset(vao[:, :, :, DV:DV + 1], 1.0)
            return vao

        def attention(ph, aps, qt, kt, vg, masked, nm):
            """Local K/V -> normalized ctx_sb [P, ND, TOK] bf16."""
            ctx_sb = pers.tile([P, ND, TOK], bf16, tag="ctxs", name=f"ctx_{nm}")
            for p in range(HP):
                # cps[h][qh]: [DV+1, 512] accumulated over key blocks
                cps = [[aps.tile([DV + 1, 512], f32, tag="ctxps", bufs=4,
                                 name=f"cps_{nm}{p}{h}{qh}") for qh in range(2)]
                       for h in range(2)]
                for kb in range(NKB):
                    for qh in range(2):
                        if masked:
                            if qh == 0 and kb > 3:
                                continue
                            qo = max(0, kb * P - qh * 512)
                            dslab = (kb < 4) if qh == 0 else (kb >= 4)
                        else:
                            qo = 0
                            dslab = False
                        q0 = qh * 512
                        es = ph.tile([P, 2, 512], bf16, tag="es", bufs=3,
                                     name=f"es_{nm}{p}{kb}{qh}")
                        for h in range(2):
                            sc = aps.tile([P, 512], f32, tag="sc", bufs=2,
                                          name=f"sc_{nm}{p}{kb}{qh}{h}")
                            nc.tensor.matmul(
                                sc[:, qo:],
                                kt[h * DK:(h + 1) * DK, p,
                                   kb * P:(kb + 1) * P],
                                qt[h * DK:(h + 1) * DK, p, q0 + qo:q0 + 512],
                                start=True, stop=True)
                            nc.scalar.activation(
                                es[:, h, qo:], sc[:, qo:],
                                AF.Exp, scale=1.0 / math.sqrt(DK))
                        if dslab:
                            nc.vector.tensor_tensor(
                                es[:, :, qo:qo + P], es[:, :, qo:qo + P],
                                smask_sb[:, kb, None, :].to_broadcast([P, 2, P]),
                                OP.mult)
                        last_kb = (3 if (masked and qh == 0) else NKB - 1)
                        for h in range(2):
                            nc.tensor.matmul(
                                cps[h][qh][:, qo:], vg[:, kb, 2 * p + h, :],
                                es[:, h, qo:], start=(kb == 0),
                                stop=(kb == last_kb))
                for h in range(2):
                    for qh in range(2):
                        rec = pers.tile([1, 512], f32r, tag="rec", bufs=2,
                                        name=f"rec_{nm}{p}{h}{qh}")
                        with nc.allow_low_precision(reason="f32r softmax denom"):
                            nc.vector.reciprocal(rec[:], cps[h][qh][DV:DV + 1, :])
                        bc = aps.tile([P, 512], f32, tag="sc", bufs=2,
                                      name=f"bc_{nm}{p}{h}{qh}")
                        nc.tensor.matmul(bc[:], ones_row[:], rec[:],
                                         start=True, stop=True)
                        cslc = ctx_sb[h * DV:(h + 1) * DV, p,
                                      qh * 512:(qh + 1) * 512]
                        nc.vector.tensor_copy(cslc, cps[h][qh][0:DV, :])
                        nc.vector.tensor_tensor(cslc, cslc, bc[0:DV, :],
                                                OP.mult)
            return ctx_sb

        def ln_apply(xn, nm):
            """In-place layernorm of xn across the DM (partition-tiled) axis."""
            with tc.tile_pool(name=f"lps_{nm}", bufs=1, space="PSUM") as lps:
                ssum = [lps.tile([1, 512], f32, tag=f"stsum{h}",
                                 name=f"ssum_{nm}{h}") for h in range(2)]
                ssq = [lps.tile([1, 512], f32, tag=f"stsq{h}",
                                name=f"ssq_{nm}{h}") for h in range(2)]
                for m in range(ND):
                    sq = pers.tile([P, TOK], f32r, tag="sq", bufs=2,
                                   name=f"sq_{nm}{m}")
                    nc.scalar.square(sq[:], xn[:, m, :])
                    for half in range(2):
                        cs = slice(half * 512, (half + 1) * 512)
                        nc.tensor.matmul(ssum[half][:], ones_col[:],
                                         xn[:, m, cs],
                                         start=(m == 0), stop=(m == ND - 1))
                        nc.tensor.matmul(ssq[half][:], ones_col[:], sq[:, cs],
                                         start=(m == 0), stop=(m == ND - 1))
                mean = pers.tile([1, TOK], f32r, tag="mean", name=f"mean_{nm}")
                es2 = pers.tile([1, TOK], f32, tag="lnt", bufs=2,
                                name=f"es2_{nm}")
                for half in range(2):
                    cs = slice(half * 512, (half + 1) * 512)
                    nc.vector.tensor_scalar_mul(mean[:, cs], ssum[half][:],
                                                1.0 / DM)
                    nc.vector.tensor_scalar_mul(es2[:, cs], ssq[half][:],
                                                1.0 / DM)
                msq = pers.tile([1, TOK], f32, tag="lnt", bufs=2,
                                name=f"msq_{nm}")
                nc.scalar.square(msq[:], mean[:])
                # var computed in place over es2
                nc.vector.tensor_tensor(es2[:], es2[:], msq[:], OP.subtract)
                sS = pers.tile([1, TOK], f32r, tag="lnt", bufs=2,
                               name=f"sS_{nm}")
                nc.scalar.activation(sS[:], es2[:], AF.Abs_reciprocal_sqrt,
                                     bias=eps_t[:])
                Mbs = pers.tile([P, TOK], f32, tag="Mbs", name=f"Mbs_{nm}")
                Sbs = pers.tile([P, TOK], f32, tag="Sbs", name=f"Sbs_{nm}")
                for half in range(2):
                    cs = slice(half * 512, (half + 1) * 512)
                    Mb = lps.tile([P, 512], f32, tag="Mb", bufs=2,
                                  name=f"Mb_{nm}{half}")
                    nc.tensor.matmul(Mb[:], ones_row[:], mean[:, cs],
                                     start=True, stop=True)
                    nc.scalar.copy(Mbs[:, cs], Mb[:])
                    Sb = lps.tile([P, 512], f32, tag="Mb", bufs=2,
                                  name=f"Sb_{nm}{half}")
                    nc.tensor.matmul(Sb[:], ones_row[:], sS[:, cs],
                                     start=True, stop=True)
                    nc.scalar.copy(Sbs[:, cs], Sb[:])
                for m in range(ND):
                    nc.vector.tensor_tensor(xn[:, m, :], xn[:, m, :], Mbs[:],
                                            OP.subtract)
                for m in range(ND):
                    nc.vector.tensor_tensor(xn[:, m, :], xn[:, m, :], Sbs[:],
                                            OP.mult)
            x_cur = xn

        def wo_add(ph, aps, wo2d, ctx_sb, nm):
            """Wo matmuls (bf16) + in-place residual add into x_cur."""
            whs = [load_w8b(ph, wo2d, half, f"o{nm}{half}") for half in range(2)]
            for m in range(ND):
                half, mm = divmod(m, 4)
                for h2 in range(2):
                    cs = slice(h2 * 512, (h2 + 1) * 512)
                    ps = aps.tile([P, 512], f32, tag="proj", bufs=2,
                                  name=f"wops_{nm}{m}{h2}")
                    for v in range(ND):
                        nc.tensor.matmul(
                            ps[:], whs[half][:, v, mm * P:(mm + 1) * P],
                            ctx_sb[:, v, cs], start=(v == 0), stop=(v == ND - 1))
                    nc.vector.tensor_tensor(x_cur[:, m, cs], ps[:],
                                            x_cur[:, m, cs], OP.add)

        for l in range(L):
            # Self sublayer
            with tc.tile_pool(name=f"ph1_{l}", bufs=1) as ph, \
                 tc.tile_pool(name=f"ps1_{l}", bufs=1, space="PSUM") as aps:
                xb = cast_xb(ph, f"s{l}")
                kt_s = kq_proj(ph, aps, xb, wv2d(l, "self_Wk"), f"ks{l}")
                vg_s = v_proj(ph, aps, xb, wv2d(l, "self_Wv"), f"s{l}")
                qt = kq_proj(ph, aps, xb, wv2d(l, "self_Wq"), f"qs{l}")
                ctx = attention(ph, aps, qt, kt_s, vg_s, self_causal, f"s{l}")
                wo_add(ph, aps, wv2d(l, "self_Wo"), ctx, f"s{l}")
            ln_apply(x_cur, f"s{l}")

            # cross sublayer (K/V from the static enc)
            with tc.tile_pool(name=f"ph4_{l}", bufs=1) as ph2, \
                 tc.tile_pool(name=f"ps4_{l}", bufs=1, space="PSUM") as aps2:
                kt_c = kq_proj(ph2, aps2, enc_sb, wv2d(l, "cross_Wk"), f"kc{l}")
                vg_c = v_proj(ph2, aps2, enc_sb, wv2d(l, "cross_Wv"), f"c{l}")
                xb = cast_xb(ph2, f"c{l}")
                qtc = kq_proj(ph2, aps2, xb, wv2d(l, "cross_Wq"), f"qc{l}")
                ctx = attention(ph2, aps2, qtc, kt_c, vg_c, False, f"c{l}")
                wo_add(ph2, aps2, wv2d(l, "cross_Wo"), ctx, f"c{l}")
            ln_apply(x_cur, f"c{l}")

            # FFN
            with tc.tile_pool(name=f"ph6_{l}", bufs=1) as ph:
                xb = cast_xb(ph, f"f{l}")
                h_sb = ph.tile([P, NF, TOK], bf16, tag="h", name=f"h_{l}")
                with tc.tile_pool(name=f"ps6_{l}", bufs=1, space="PSUM") as pools:
                    w1r = wv2d(l, "ffn_W1").rearrange(
                        "(o p x) c -> p o (x c)", p=P, x=4)
                    for c in range(DFF // 512):
                        w1c = ph.tile([P, ND, 512], bf16, tag="whb", bufs=2,
                                      name=f"w1c_{l}{c}")
                        if c == 0:
                            # split first chunk across queues to cut latency
                            for d in range(ND):
                                nc.sync.dma_start(
                                    w1c[:, d, :], w1r[:, d, 0:512])
                        else:
                            nc.sync.dma_start(
                                w1c[:], w1r[:, :, c * 512:(c + 1) * 512])
                        for ft in range(4):
                            for half in range(2):
                                cs = slice(half * 512, (half + 1) * 512)
                                ps = pools.tile([P, 512], f32, tag="hps",
                                                bufs=4,
                                                name=f"hps_{l}{c}{ft}{half}")
                                for d in range(ND):
                                    nc.tensor.matmul(
                                        ps[:], w1c[:, d, ft * P:(ft + 1) * P],
                                        xb[:, d, cs],
                                        start=(d == 0), stop=(d == ND - 1))
                                nc.scalar.activation(
                                    h_sb[:, c * 4 + ft, cs], ps[:], AF.Relu)
                with tc.tile_pool(name=f"ps7_{l}", bufs=1, space="PSUM") as pools:
                    w2r = wv2d(l, "ffn_W2").rearrange("(f p) m -> p f m", p=P)
                    for half in range(2):
                        cs = slice(half * 512, (half + 1) * 512)
                        yps = [pools.tile([P, 512], f32, tag=f"y{m}",
                                          name=f"yps_{l}{m}{half}")
                               for m in range(ND)]
                        for f in range(NF):
                            w2f = ph.tile([P, DM], bf16, tag="w2f", bufs=3,
                                          name=f"w2f_{l}{half}{f}")
                            nc.sync.dma_start(w2f[:], w2r[:, f, :])
                            for m in range(ND):
                                nc.tensor.matmul(
                                    yps[m][:], w2f[:, m * P:(m + 1) * P],
                                    h_sb[:, f, cs],
                                    start=(f == 0), stop=(f == NF - 1))
                        for m in range(ND):
                            nc.vector.tensor_tensor(x_cur[:, m, cs], yps[m][:],
                                                    x_cur[:, m, cs], OP.add)
                ln_apply(x_cur, f"f{l}")

        yre = yT_ext.rearrange("(o p) t -> p o t", p=P).bitcast(f32r)
        for m in range(ND):
            nc.sync.dma_start(yre[:, m, :], x_cur[:, m, :])

    nc.compile()
    return nc


def _get_built(self_causal=True):
    if self_causal not in _BUILT:
        _BUILT[self_causal] = _build(self_causal=self_causal)
    return _BUILT[self_causal]


def _pack_weights(inputs):
    """Pack all weights (bf16) into per-core blob chunks [8, L*CROWS, 1024]."""
    blob = np.empty((L, LROWS, 1024), dtype=ml_dtypes.bfloat16)
    for l in range(L):
        for name in WORDER:
            w = np.asarray(inputs[name][l], dtype=np.float32)
            r0 = WOFF[name]
            nrows = w.size // 1024
            blob[l, r0:r0 + nrows] = w.astype(ml_dtypes.bfloat16).reshape(
                nrows, 1024)
    # core c gets rows [c*CROWS:(c+1)*CROWS) of each layer's blob
    return np.ascontiguousarray(
        blob.reshape(L, 8, CROWS, 1024).transpose(1, 0, 2, 3).reshape(
            8, L * CROWS, 1024))


def _host_shard(inputs):
    """Build per-core input maps from full inputs."""
    dec = np.asarray(inputs["dec_inputs"], dtype=np.float32)
    enc = np.asarray(inputs["enc_outputs"], dtype=np.float32)
    smask_full = np.asarray(inputs["dec_self_attn_mask"]).astype(bool)
    cmask = np.asarray(inputs["dec_enc_attn_mask"]).astype(bool)
    assert not cmask.any(), "kernel assumes open cross-attention mask"

    wchunks = _pack_weights(inputs)
    self_causal = smask_full.any()

    in_maps = []
    for core in range(8):
        b = core // 2
        xT = np.ascontiguousarray(dec[b].T).astype(ml_dtypes.bfloat16)
        encT = np.ascontiguousarray(enc[b].T).astype(ml_dtypes.bfloat16)
        sm = np.ones((NKB, P, P), dtype=np.float32)
        mb = smask_full[b]
        if self_causal:
            for kb in range(NKB):
                blk = mb[kb * P:(kb + 1) * P, kb * P:(kb + 1) * P]  # [q, k]
                sm[kb] = (~blk.T).astype(np.float32)                # [k, q]
                for qb in range(NKB):
                    bj = mb[qb * P:(qb + 1) * P, kb * P:(kb + 1) * P]
                    if qb < kb:
                        assert bj.all(), "skipped block not fully masked"
                    elif qb > kb:
                        assert not bj.any(), \
                            "unmasked block outside computed window"
        in_map = {"xT": xT, "encT": encT,
                  "smask": sm.astype(ml_dtypes.bfloat16),
                  "wchunk": wchunks[core]}
        in_maps.append(in_map)
    return in_maps, self_causal


def kernel(**inputs):
    from concourse.bass_utils import run_bass_kernel_spmd

    in_maps, self_causal = _host_shard(inputs)
    nc = _get_built(self_causal)
    res = run_bass_kernel_spmd(nc, in_maps, core_ids=list(range(8)))
    out = np.empty((B, T, DM), dtype=np.float32)
    for b in range(B):
        out[b] = res.results[2 * b]["yT"].T
    return out
